# revision 9
# baseline (speedup 1.0000x reference)
"""Mat2Twist TRN2 kernel v2: fp16 I/O, |w|-normalized axis, arctan theta.

Math per matrix R (fp32 internal compute, fp16 storage):
  w  = [R21-R12, R02-R20, R10-R01]          (|w| = 2 sin theta)
  s2 = w0^2+w1^2+w2^2;  u = rsqrt(s2) = 1/|w|
  tr = R00+R11+R22;     cot(theta) = ((tr-1)/2) / sin(theta) = (tr-1)*u
  theta = pi/2 + arctan(-(tr-1)*u)
  out   = theta * w/|w| = ((arctan(-(tr-1)*u) + pi/2) * u) * w

fp16 end-to-end halves HBM traffic vs f32 (tolerance is 2e-2 rel to
absmax; this chain measures ~3e-3). Host packs the input fp16
component-major per chunk (PERM order) so every on-chip op and DMA is
unit-stride; host casts the fp16 output back to f32.

ACT table-set discipline: Square is a filler in every set; Rsqrt lives
only in reciprocal_sqrt_and_small, Arctan only in sigmoid/trig sets.
Chunks are processed in supergroups: all Rsqrt-set work (stage A) of a
group precedes its Arctan batch (stage B), bounding table reloads.

DMA: input chunks stream on the sync-engine HWDGE ring; outputs leave
via GpSimd SWDGE so input prefetch is never head-of-line blocked behind
an output DMA that waits on compute.
"""

import numpy as np

import concourse.bass as bass
import concourse.mybir as mybir
from concourse.tile import TileContext
from concourse.bass_utils import run_bass_kernel_spmd

B = 4194304
NCORES = 8
P = 128
N_C = B // NCORES        # 524288 matrices per core
MPP = N_C // P           # 4096 matrices per partition

# chunk sizes (matrices per partition) and supergroup split
MS = [512, 1024, 1024, 1024, 256, 256]
GROUPS = [[0, 1, 2, 3], [4, 5]]
assert sum(MS) == MPP

# component order in DRAM (flat 3x3 index): minuends, subtrahends, diagonal
PERM = [7, 2, 3, 5, 6, 1, 0, 4, 8]

F16 = mybir.dt.float16
F32 = mybir.dt.float32
ACT = mybir.ActivationFunctionType
ALU = mybir.AluOpType
PI_2 = float(np.pi / 2.0)
MAXM = max(MS)
OFFS = [0]
for m in MS[:-1]:
    OFFS.append(OFFS[-1] + m)

_TRACE = False
_PROF = {}


def _split_multi_waits(nc):
    """This container's walrus build rejects >1 sem-wait per instruction
    ("Too many sync wait commands"); hoist extras onto preceding NOPs."""
    for f in nc.m.functions:
        for blk in f.blocks:
            il = blk.instructions
            new = []
            for ins in il:
                si = ins.sync_info
                if si is not None and si.on_wait is not None and len(si.on_wait) > 1:
                    waits = list(si.on_wait)
                    for j, w in enumerate(waits[:-1]):
                        nop = mybir.InstNoOp(name=f"{ins.name}-ws{j}", engine=ins.engine)
                        nop.sync_info = mybir.SyncInfo(on_wait=[w], on_update=[])
                        new.append(nop)
                    ins.sync_info = mybir.SyncInfo(
                        on_wait=[waits[-1]], on_update=list(si.on_update or [])
                    )
                new.append(ins)
            il[:] = new


def _build_kernel():
    nc = bass.Bass()
    x_in = nc.dram_tensor("mat_in", [N_C * 9], F16, kind="ExternalInput")
    y_out = nc.dram_tensor("twist_out", [N_C * 3], F16, kind="ExternalOutput")

    with TileContext(nc) as tc:
        with tc.tile_pool(name="io", bufs=3) as io_pool, \
             tc.tile_pool(name="oo", bufs=6) as oo_pool, \
             tc.tile_pool(name="tw2", bufs=2) as w2_pool, \
             tc.tile_pool(name="tsm", bufs=2) as sm_pool, \
             tc.tile_pool(name="tgr", bufs=6) as gr_pool, \
             tc.tile_pool(name="tsb", bufs=3) as sb_pool, \
             tc.tile_pool(name="tgt", bufs=4) as gt_pool:

            state = {}

            def stageA(ci, ln_gate=None):
                off, m = OFFS[ci], MS[ci]
                tile = io_pool.tile([P, 9 * MAXM], F16, tag="in", name=f"in{ci}")[:, : 9 * m]
                src = x_in[off * P * 9 : (off + m) * P * 9].rearrange(
                    "(p n) -> p n", p=P
                )
                nc.sync.dma_start(out=tile, in_=src)

                # w (unnormalized axis) lives in the output tile
                ot = oo_pool.tile([P, 3 * MAXM], F16, tag="out", name=f"out{ci}")[:, : 3 * m]
                nc.vector.tensor_sub(
                    out=ot, in0=tile[:, 0 : 3 * m], in1=tile[:, 3 * m : 6 * m]
                )

                w2 = w2_pool.tile([P, 3 * MAXM], F16, tag="w2", name=f"w2{ci}")[:, : 3 * m]
                nc.scalar.activation(w2, ot, ACT.Square)  # filler fn: no table load

                s2 = sm_pool.tile([P, MAXM], F16, tag="s2", name=f"s2{ci}")[:, :m]
                nc.vector.tensor_add(out=s2, in0=w2[:, 0:m], in1=w2[:, m : 2 * m])
                nc.vector.tensor_add(out=s2, in0=s2, in1=w2[:, 2 * m : 3 * m])

                lg = sm_pool.tile([P, MAXM], F16, tag="lg", name=f"lg{ci}")[:, :m]
                # ln_gate (value 1.0) makes this Ln depend on the previous
                # group's last Arctan so the scheduler cannot interleave
                # table sets.
                nc.scalar.activation(
                    lg, s2, ACT.Ln,
                    scale=1.0 if ln_gate is None else ln_gate[:, 0:1],
                )
                u = gr_pool.tile([P, MAXM], F16, tag="u", name=f"u{ci}")[:, :m]
                nc.scalar.activation(u, lg, ACT.Exp, scale=-0.5)  # 1/|w|

                tr = sm_pool.tile([P, MAXM], F16, tag="tr", name=f"tr{ci}")[:, :m]
                nc.vector.tensor_add(
                    out=tr, in0=tile[:, 6 * m : 7 * m], in1=tile[:, 7 * m : 8 * m]
                )
                nc.vector.tensor_add(out=tr, in0=tr, in1=tile[:, 8 * m : 9 * m])

                tm = gr_pool.tile([P, MAXM], F16, tag="tm", name=f"tm{ci}")[:, :m]
                nc.vector.scalar_tensor_tensor(
                    out=tm, in0=tr, scalar=1.0, in1=u,
                    op0=ALU.subtract, op1=ALU.mult,
                )  # (tr-1)*u = cot(theta)
                state[ci] = (ot, u, tm)

            def stageB(ci, at_gate):
                off, m = OFFS[ci], MS[ci]
                ot, u, tm = state.pop(ci)
                at = sb_pool.tile([P, MAXM], F16, tag="at", name=f"at{ci}")[:, :m]
                # at_gate (value -1.0) makes the Arctan depend on this
                # group's last Exp: all lnexp-set work precedes the batch.
                nc.scalar.activation(at, tm, ACT.Arctan, scale=at_gate[:, 0:1])

                sc = sb_pool.tile([P, MAXM], F16, tag="sc", name=f"sc{ci}")[:, :m]
                nc.vector.scalar_tensor_tensor(
                    out=sc, in0=at, scalar=PI_2, in1=u,
                    op0=ALU.add, op1=ALU.mult,
                )  # theta/|w|

                for k in range(3):
                    blk = ot[:, k * m : (k + 1) * m]
                    nc.vector.tensor_mul(out=blk, in0=sc, in1=blk)
                dst = y_out[off * P * 3 : (off + m) * P * 3].rearrange(
                    "(p n) -> p n", p=P
                )
                nc.gpsimd.dma_start(out=dst, in_=ot)
                return at

            ln_gate = None
            for gi, grp in enumerate(GROUPS):
                for ci in grp:
                    stageA(ci, ln_gate)
                u_last = state[grp[-1]][1]
                at_gate = gt_pool.tile([P, 1], F32, tag=f"ga{gi}", name=f"ga{gi}")
                nc.vector.tensor_scalar(
                    out=at_gate, in0=u_last[:, 0:1], scalar1=0.0, scalar2=-1.0,
                    op0=ALU.mult, op1=ALU.add,
                )
                last_at = None
                for ci in grp:
                    last_at = stageB(ci, at_gate)
                if gi + 1 < len(GROUPS):
                    ln_gate = gt_pool.tile([P, 1], F32, tag=f"gl{gi}", name=f"gl{gi}")
                    nc.vector.tensor_scalar(
                        out=ln_gate, in0=last_at[:, 0:1], scalar1=0.0, scalar2=1.0,
                        op0=ALU.mult, op1=ALU.add,
                    )

    _split_multi_waits(nc)
    return nc


_NC_CACHE = []


def _host_pack(mat_batch: np.ndarray) -> np.ndarray:
    """[B,3,3] f32 -> [NCORES, N_C*9] fp16 tile-major/component-major PERM."""
    flat = np.ascontiguousarray(mat_batch, dtype=np.float32).reshape(
        NCORES, N_C, 9
    ).astype(np.float16)
    out = np.empty((NCORES, N_C * 9), np.float16)
    for m, off in zip(MS, OFFS):
        chunk = flat[:, off * P : (off + m) * P, :].reshape(NCORES, P, m, 9)
        sz = P * m * 9
        out[:, off * P * 9 : off * P * 9 + sz] = (
            chunk.transpose(0, 1, 3, 2)[:, :, PERM, :].reshape(NCORES, sz)
        )
    return out


def _host_unpack(res_list) -> np.ndarray:
    out = np.empty((B, 3), np.float32)
    o = out.reshape(NCORES, N_C, 3)
    for i, r in enumerate(res_list):
        y = r["twist_out"]
        for m, off in zip(MS, OFFS):
            sz = P * m * 3
            blk = y[off * P * 3 : off * P * 3 + sz].reshape(P, 3, m)
            o[i, off * P : (off + m) * P, :] = blk.transpose(0, 2, 1).reshape(
                P * m, 3
            )
    return out


def kernel(mat_batch: np.ndarray) -> np.ndarray:
    if not _NC_CACHE:
        _NC_CACHE.append(_build_kernel())
    nc = _NC_CACHE[0]

    packed = _host_pack(mat_batch)
    in_maps = [{"mat_in": packed[i]} for i in range(NCORES)]
    res = run_bass_kernel_spmd(
        nc, in_maps, core_ids=list(range(NCORES)), trace=_TRACE
    )
    if _TRACE:
        _PROF["exec_time_ns"] = res.exec_time_ns
        _PROF["profile_json"] = res.profile_json
        _PROF["insts_and_trace"] = res.instructions_and_trace
    return _host_unpack(res.results)
